# revision 7
# baseline (speedup 1.0000x reference)
"""Bidirectional 2-layer LSTM (shared weights) on 8 Trainium2 cores.

Strategy: the two directions are independent scans over T=256 steps with
identical weights, so cores run one direction each (fwd on even cores,
rev on odd cores; extra core pairs run redundant replicas). On a core the
two layers are software-pipelined with a one-step lag — the emission
order per slot is [L0(t+1) matmuls, L1(t) matmuls, L0(t+1) cell, L1(t)
cell] so the tensor engine always has the other layer's matmuls to run
while a cell chain (ACT/DVE) completes, keeping PE busy and HAM warm.

Matmul layout: gates[B=128, 4H] = lhsT.T @ rhs with lhsT = x_t^T / h^T
(stationary, [K,128]) and rhs = W^T (moving, N=512 per PSUM bank), all in
fp32r (fp32 with 12-bit mantissa, full rate on the PE). The bias enters
as a K=1 matmul against a ones row (keeps it off the cell critical
chain). Gate columns are permuted to [i|f|o|g] per 512-block so sigmoid
covers a strided 4x384 region in one ACT instruction. h^T is produced by
4 PE transposes into the already-consumed PSUM gate regions, then one
ACT copy (cast to fp32r) back to SBUF, where it serves as the next
step's stationary operand and as layer 1's input-projection operand.
"""
import sys

sys.path.insert(0, "/opt/trn_rl_repo")

import numpy as np
import concourse.bass as bass
import concourse.bacc as bacc
import concourse.tile as tile
import concourse.mybir as mybir
from concourse import bass_utils

B, T, IN, H, OUT = 128, 256, 256, 512, 256
G = 4 * H
NB = G // 512          # 4 gate-column blocks (one PSUM bank each)
KH = H // 128          # 4 contraction tiles for h
KX = IN // 128         # 2 contraction tiles for x
F32 = mybir.dt.float32
F32R = mybir.dt.float32r
AF = mybir.ActivationFunctionType

N_CORES = 8


def gate_perm():
    # original gate order along 4H: [i(512) | f(512) | g(512) | o(512)]
    # new order: per hidden block nb of 128: [i_nb | f_nb | o_nb | g_nb]
    idx = []
    for nb in range(NB):
        s = nb * 128
        idx += list(range(0 + s, 128 + s))        # i
        idx += list(range(512 + s, 640 + s))      # f
        idx += list(range(1536 + s, 1664 + s))    # o
        idx += list(range(1024 + s, 1152 + s))    # g
    return np.array(idx)


def build(t_steps=T, unroll=5, n_cores=N_CORES):
    nc = bacc.Bacc("TRN2", target_bir_lowering=False, debug=False,
                   num_devices=n_cores)

    def din(name, shape, dt=F32R):
        return nc.dram_tensor(name, shape, dt, kind="ExternalInput").ap()

    xT = din("xT", [t_steps, KX, 128, B])
    wih0 = din("wih0", [KX, 128, G])
    whh0 = din("whh0", [KH, 128, G])
    wih1 = din("wih1", [KH, 128, G])
    whh1 = din("whh1", [KH, 128, G])
    b0 = din("b0", [NB, 512])
    b1 = din("b1", [NB, 512])
    wfc = din("wfc", [KH, 128, OUT])
    ident = din("ident", [128, 128], F32)
    ones = din("ones", [1, 128])
    zeros = din("zeros", [128, H], F32R)
    y = nc.dram_tensor("y", [B, OUT], F32, kind="ExternalOutput").ap()

    with tile.TileContext(nc) as tc:
        with tc.tile_pool(name="wpool", bufs=1) as wp, \
             tc.tile_pool(name="state", bufs=1) as st, \
             tc.tile_pool(name="xin", bufs=3) as xp, \
             tc.tile_pool(name="cellp", bufs=2) as cp, \
             tc.tile_pool(name="gpool", bufs=2, space="PSUM") as gp:

            s_wih0 = wp.tile([128, KX, G], F32R)
            nc.sync.dma_start(out=s_wih0, in_=wih0.rearrange("k p g -> p k g"))
            s_whh0 = wp.tile([128, KH, G], F32R)
            nc.sync.dma_start(out=s_whh0, in_=whh0.rearrange("k p g -> p k g"))
            s_wih1 = wp.tile([128, KH, G], F32R)
            nc.sync.dma_start(out=s_wih1, in_=wih1.rearrange("k p g -> p k g"))
            s_whh1 = wp.tile([128, KH, G], F32R)
            nc.sync.dma_start(out=s_whh1, in_=whh1.rearrange("k p g -> p k g"))
            s_b0 = wp.tile([1, NB, 512], F32R)
            nc.sync.dma_start(out=s_b0, in_=b0.rearrange("nb c -> (nb c)")[None, :]
                              .rearrange("o (nb c) -> o nb c", c=512))
            s_b1 = wp.tile([1, NB, 512], F32R)
            nc.sync.dma_start(out=s_b1, in_=b1.rearrange("nb c -> (nb c)")[None, :]
                              .rearrange("o (nb c) -> o nb c", c=512))
            s_ones = wp.tile([1, 128], F32R)
            nc.sync.dma_start(out=s_ones, in_=ones)
            s_wfc = wp.tile([128, KH, OUT], F32R)
            nc.sync.dma_start(out=s_wfc, in_=wfc.rearrange("k p o -> p k o"))
            s_id = wp.tile([128, 128], F32)
            nc.sync.dma_start(out=s_id, in_=ident)

            hT0 = st.tile([128, H], F32R)
            hT1 = st.tile([128, H], F32R)
            c0 = st.tile([128, H], F32)
            c1 = st.tile([128, H], F32)
            for t_ in (hT0, hT1):
                nc.sync.dma_start(out=t_, in_=zeros)
            for t_ in (c0, c1):
                nc.vector.memset(t_, 0.0)

            # prewarm the sigmoid/tanh ACT table set so the table load is
            # hoisted out of the step loop
            dummy = st.tile([128, 1], F32)
            nc.vector.memset(dummy, 0.0)
            nc.scalar.activation(out=dummy, in_=dummy, func=AF.Sigmoid)

            def emit_mms(xparts, kx, wih_s, whh_s, bias_s, hT):
                gt = gp.tile([128, NB, 512], F32, tag="gt")
                for nb in range(NB):
                    nc.tensor.matmul(gt[:, nb, :], s_ones, bias_s[:, nb, :],
                                     start=True, stop=False)
                    for k in range(kx):
                        nc.tensor.matmul(
                            gt[:, nb, :], xparts[:, k, :],
                            wih_s[:, k, nb * 512:(nb + 1) * 512],
                            start=False, stop=False)
                    for k in range(KH):
                        nc.tensor.matmul(
                            gt[:, nb, :], hT[:, k * 128:(k + 1) * 128],
                            whh_s[:, k, nb * 512:(nb + 1) * 512],
                            start=False, stop=(k == KH - 1))
                return gt

            def emit_cell(gt, hT, c):
                sig = cp.tile([128, NB, 384], F32, tag="sig")
                nc.scalar.activation(out=sig, in_=gt[:, :, 0:384], func=AF.Sigmoid)
                g = cp.tile([128, NB, 128], F32, tag="g")
                nc.scalar.activation(out=g, in_=gt[:, :, 384:512], func=AF.Tanh)
                cv = c.rearrange("p (nb h) -> p nb h", h=128)
                fc_ = cp.tile([128, NB, 128], F32, tag="fc")
                nc.vector.tensor_mul(fc_, sig[:, :, 128:256], cv)
                ig = cp.tile([128, NB, 128], F32, tag="ig")
                nc.vector.tensor_mul(ig, sig[:, :, 0:128], g)
                nc.vector.tensor_add(cv, fc_, ig)
                tcn = cp.tile([128, NB, 128], F32, tag="tcn")
                nc.scalar.activation(out=tcn, in_=cv, func=AF.Tanh)
                h = cp.tile([128, NB, 128], F32, tag="h")
                nc.vector.tensor_mul(h, sig[:, :, 256:384], tcn)
                for k in range(KH):
                    nc.tensor.transpose(gt[:, k, 0:128], h[:, k, :], s_id)
                hTv = hT.rearrange("p (k h) -> p k h", h=128)
                nc.scalar.activation(out=hTv, in_=gt[:, :, 0:128], func=AF.Copy)

            hT0v = hT0.rearrange("p (k h) -> p k h", h=128)

            def l0_step(t_expr):
                xt = xp.tile([128, KX, B], F32R, tag="xt")
                nc.sync.dma_start(
                    out=xt,
                    in_=xT[bass.ds(t_expr, 1)].rearrange("o k p b -> p (o k) b"))
                gt = emit_mms(xt, KX, s_wih0, s_whh0, s_b0, hT0)
                return gt

            def slot(t_l1):
                # pipelined slot: L0 runs step t_l1+1, L1 runs step t_l1
                gt0 = l0_step(t_l1 + 1)
                gt1 = emit_mms(hT0v, KH, s_wih1, s_whh1, s_b1, hT1)
                emit_cell(gt0, hT0, c0)
                emit_cell(gt1, hT1, c1)

            # prologue: L0 step 0
            gt = l0_step(0)
            emit_cell(gt, hT0, c0)

            loop_steps = t_steps - 1
            n_iter = loop_steps // unroll
            if n_iter > 1:
                with tc.For_i(0, n_iter,
                              hint_engines=(mybir.EngineType.PE,)) as iv:
                    for u in range(unroll):
                        slot(iv * unroll + u)
            else:
                for t_i in range(n_iter * unroll):
                    slot(t_i)
            for t_i in range(n_iter * unroll, loop_steps):
                slot(t_i)

            # epilogue: L1 step t_steps-1
            gt1 = emit_mms(hT0v, KH, s_wih1, s_whh1, s_b1, hT1)
            emit_cell(gt1, hT1, c1)

            # FC partial: y = h1_final @ W_fc_dir.T  ([B, OUT])
            yps = gp.tile([B, OUT], F32, tag="gt")
            for k in range(KH):
                nc.tensor.matmul(yps, hT1[:, k * 128:(k + 1) * 128],
                                 s_wfc[:, k, :],
                                 start=(k == 0), stop=(k == KH - 1))
            ysb = st.tile([B, OUT], F32)
            nc.scalar.activation(out=ysb, in_=yps, func=AF.Copy)
            nc.sync.dma_start(out=y, in_=ysb)

    nc.compile()
    return nc


def host_inputs(x, W_ih0, W_hh0, b_ih0, b_hh0, W_ih1, W_hh1, b_ih1, b_hh1,
                W_fc, b_fc, t_steps=T):
    """Build the per-core in_maps (fwd on even cores, rev on odd cores)."""
    perm = gate_perm()

    def wprep(w, ktiles):
        # w: [4H, K_in] -> permuted-gate W^T as [ktiles, 128, G]
        wt = np.ascontiguousarray(w[perm].T.astype(np.float32))
        return np.ascontiguousarray(wt.reshape(ktiles, 128, G))

    wih0 = wprep(W_ih0, KX)
    whh0 = wprep(W_hh0, KH)
    wih1 = wprep(W_ih1, KH)
    whh1 = wprep(W_hh1, KH)
    b0 = np.ascontiguousarray((b_ih0 + b_hh0)[perm].reshape(NB, 512)
                              .astype(np.float32))
    b1 = np.ascontiguousarray((b_ih1 + b_hh1)[perm].reshape(NB, 512)
                              .astype(np.float32))
    ident = np.eye(128, dtype=np.float32)

    # x: [B, T, IN] -> x^T per step [T, KX, 128, B]
    xTf = np.ascontiguousarray(
        x.transpose(1, 2, 0).reshape(t_steps, KX, 128, B).astype(np.float32))
    rev_idx = (-np.arange(t_steps)) % t_steps
    xTr = np.ascontiguousarray(xTf[rev_idx])

    wfc_f = np.ascontiguousarray(
        W_fc[:, :H].T.astype(np.float32).reshape(KH, 128, OUT))
    wfc_r = np.ascontiguousarray(
        W_fc[:, H:].T.astype(np.float32).reshape(KH, 128, OUT))

    common = dict(wih0=wih0, whh0=whh0, wih1=wih1, whh1=whh1,
                  b0=b0, b1=b1, ident=ident,
                  ones=np.ones((1, 128), dtype=np.float32),
                  zeros=np.zeros((128, H), dtype=np.float32))
    fwd = dict(common, xT=xTf, wfc=wfc_f)
    rev = dict(common, xT=xTr, wfc=wfc_r)
    return [dict(fwd) if i % 2 == 0 else dict(rev) for i in range(N_CORES)]


_NC_CACHE = {}


def kernel(x, W_ih0, W_hh0, b_ih0, b_hh0, W_ih1, W_hh1, b_ih1, b_hh1,
           W_fc, b_fc):
    x = np.asarray(x, dtype=np.float32)
    args = [np.asarray(a, dtype=np.float32) for a in
            (W_ih0, W_hh0, b_ih0, b_hh0, W_ih1, W_hh1, b_ih1, b_hh1,
             W_fc, b_fc)]
    in_maps = host_inputs(x, *args)
    key = (T,)
    if key not in _NC_CACHE:
        _NC_CACHE[key] = build(T)
    nc = _NC_CACHE[key]
    res = bass_utils.run_bass_kernel_spmd(nc, in_maps,
                                          core_ids=list(range(N_CORES)))
    y = res.results[0]["y"] + res.results[1]["y"] + args[9][None, :]
    return y.astype(np.float32)


# revision 12
# speedup vs baseline: 15.8350x; 15.8350x over previous
"""Bidirectional 2-layer LSTM (shared weights) on 8 Trainium2 cores.

Strategy: the two directions are independent scans over T=256 steps with
identical weights, so cores run one direction each (fwd on even cores,
rev on odd cores; extra core pairs run redundant replicas). On a core the
two layers are software-pipelined with a one-step lag — the emission
order per slot is [L0(t+1) matmuls, L1(t) matmuls, L0(t+1) cell, L1(t)
cell] so the tensor engine always has the other layer's matmuls to run
while a cell chain (ACT/DVE) completes, keeping PE busy and HAM warm.

Matmul layout: gates[B=128, 4H] = lhsT.T @ rhs with lhsT = x_t^T / h^T
(stationary, [K,128]) and rhs = W^T (moving, N=512 per PSUM bank), all in
fp32r (fp32 with 12-bit mantissa, full rate on the PE). The bias enters
as a K=1 matmul against a ones row (keeps it off the cell critical
chain). Gate columns are permuted to [i|f|o|g] per 512-block so sigmoid
covers a strided 4x384 region in one ACT instruction. h^T is produced by
4 PE transposes into the already-consumed PSUM gate regions, then one
ACT copy (cast to fp32r) back to SBUF, where it serves as the next
step's stationary operand and as layer 1's input-projection operand.
"""
import sys

sys.path.insert(0, "/opt/trn_rl_repo")

import numpy as np
import concourse.bass as bass
import concourse.bacc as bacc
import concourse.tile as tile
import concourse.mybir as mybir
from concourse import bass_utils

B, T, IN, H, OUT = 128, 256, 256, 512, 256
G = 4 * H
NB = G // 512          # 4 gate-column blocks (one PSUM bank each)
KH = H // 128          # 4 contraction tiles for h
KX = IN // 128         # 2 contraction tiles for x
F32 = mybir.dt.float32
F32R = mybir.dt.float32r
AF = mybir.ActivationFunctionType

N_CORES = 8


def gate_perm():
    # original gate order along 4H: [i(512) | f(512) | g(512) | o(512)]
    # new order: per hidden block nb of 128: [i_nb | f_nb | o_nb | g_nb]
    idx = []
    for nb in range(NB):
        s = nb * 128
        idx += list(range(0 + s, 128 + s))        # i
        idx += list(range(512 + s, 640 + s))      # f
        idx += list(range(1536 + s, 1664 + s))    # o
        idx += list(range(1024 + s, 1152 + s))    # g
    return np.array(idx)


def build(t_steps=T, unroll=4, n_cores=N_CORES):
    assert unroll % 2 == 0 or t_steps - 1 <= unroll  # parity rotation of hT0
    nc = bacc.Bacc("TRN2", target_bir_lowering=False, debug=False,
                   num_devices=n_cores)

    def din(name, shape, dt=F32R):
        return nc.dram_tensor(name, shape, dt, kind="ExternalInput").ap()

    xT = din("xT", [t_steps, KX, 128, B])
    wih0 = din("wih0", [KX, 128, G])
    whh0 = din("whh0", [KH, 128, G])
    wih1 = din("wih1", [KH, 128, G])
    whh1 = din("whh1", [KH, 128, G])
    b0 = din("b0", [NB, 512])
    b1 = din("b1", [NB, 512])
    wfc = din("wfc", [KH, 128, OUT])
    ident = din("ident", [128, 128], F32)
    ones = din("ones", [1, 128])
    zeros = din("zeros", [128, H], F32R)
    y = nc.dram_tensor("y", [B, OUT], F32, kind="ExternalOutput").ap()

    with tile.TileContext(nc) as tc:
        with tc.tile_pool(name="wpool", bufs=1) as wp, \
             tc.tile_pool(name="state", bufs=1) as st, \
             tc.tile_pool(name="xin", bufs=3) as xp, \
             tc.tile_pool(name="cellp", bufs=2) as cp, \
             tc.tile_pool(name="gpool", bufs=2, space="PSUM") as gp:

            s_wih0 = wp.tile([128, KX, G], F32R)
            nc.sync.dma_start(out=s_wih0, in_=wih0.rearrange("k p g -> p k g"))
            s_whh0 = wp.tile([128, KH, G], F32R)
            nc.sync.dma_start(out=s_whh0, in_=whh0.rearrange("k p g -> p k g"))
            s_wih1 = wp.tile([128, KH, G], F32R)
            nc.sync.dma_start(out=s_wih1, in_=wih1.rearrange("k p g -> p k g"))
            s_whh1 = wp.tile([128, KH, G], F32R)
            nc.sync.dma_start(out=s_whh1, in_=whh1.rearrange("k p g -> p k g"))
            s_b0 = wp.tile([1, NB, 512], F32R)
            nc.sync.dma_start(out=s_b0, in_=b0.rearrange("nb c -> (nb c)")[None, :]
                              .rearrange("o (nb c) -> o nb c", c=512))
            s_b1 = wp.tile([1, NB, 512], F32R)
            nc.sync.dma_start(out=s_b1, in_=b1.rearrange("nb c -> (nb c)")[None, :]
                              .rearrange("o (nb c) -> o nb c", c=512))
            s_ones = wp.tile([1, 128], F32R)
            nc.sync.dma_start(out=s_ones, in_=ones)
            s_wfc = wp.tile([128, KH, OUT], F32R)
            nc.sync.dma_start(out=s_wfc, in_=wfc.rearrange("k p o -> p k o"))
            s_id = wp.tile([128, 128], F32)
            nc.sync.dma_start(out=s_id, in_=ident)

            # hT0 is double-buffered (parity of the step) so the h0^T copy of
            # step t+1 can overlap layer-1's matmul reads of step t
            hT0s = [st.tile([128, H], F32R, name=f"hT0{i}", tag=f"hT0{i}")
                    for i in range(2)]
            hT1 = st.tile([128, H], F32R)
            c0 = st.tile([128, H], F32)
            c1 = st.tile([128, H], F32)
            for t_ in (hT0s[0], hT0s[1], hT1):
                nc.sync.dma_start(out=t_, in_=zeros)
            for t_ in (c0, c1):
                nc.vector.memset(t_, 0.0)

            # prewarm the sigmoid/tanh ACT table set so the table load is
            # hoisted out of the step loop
            dummy = st.tile([128, 1], F32)
            nc.vector.memset(dummy, 0.0)
            nc.scalar.activation(out=dummy, in_=dummy, func=AF.Sigmoid)

            def emit_mms(xparts, kx, wih_s, whh_s, bias_s, hT):
                gt = gp.tile([128, NB, 512], F32, tag="gt")
                for nb in range(NB):
                    nc.tensor.matmul(gt[:, nb, :], s_ones, bias_s[:, nb, :],
                                     start=True, stop=False)
                    for k in range(kx):
                        nc.tensor.matmul(
                            gt[:, nb, :], xparts[:, k, :],
                            wih_s[:, k, nb * 512:(nb + 1) * 512],
                            start=False, stop=False)
                    for k in range(KH):
                        nc.tensor.matmul(
                            gt[:, nb, :], hT[:, k * 128:(k + 1) * 128],
                            whh_s[:, k, nb * 512:(nb + 1) * 512],
                            start=False, stop=(k == KH - 1))
                return gt

            def emit_cell(gt, hT, c):
                sig = cp.tile([128, NB, 384], F32, tag="sig")
                nc.scalar.activation(out=sig, in_=gt[:, :, 0:384], func=AF.Sigmoid)
                g = cp.tile([128, NB, 128], F32, tag="g")
                nc.scalar.activation(out=g, in_=gt[:, :, 384:512], func=AF.Tanh)
                cv = c.rearrange("p (nb h) -> p nb h", h=128)
                fc_ = cp.tile([128, NB, 128], F32, tag="fc")
                nc.vector.tensor_mul(fc_, sig[:, :, 128:256], cv)
                ig = cp.tile([128, NB, 128], F32, tag="ig")
                nc.vector.tensor_mul(ig, sig[:, :, 0:128], g)
                nc.vector.tensor_add(cv, fc_, ig)
                tcn = cp.tile([128, NB, 128], F32, tag="tcn")
                nc.scalar.activation(out=tcn, in_=cv, func=AF.Tanh)
                h = cp.tile([128, NB, 128], F32, tag="h")
                nc.vector.tensor_mul(h, sig[:, :, 256:384], tcn)
                for k in range(KH):
                    nc.tensor.transpose(gt[:, k, 0:128], h[:, k, :], s_id)
                hTv = hT.rearrange("p (k h) -> p k h", h=128)
                nc.scalar.activation(out=hTv, in_=gt[:, :, 0:128], func=AF.Copy)

            def l0_step(t_expr, h_read):
                xt = xp.tile([128, KX, B], F32R, tag="xt")
                nc.sync.dma_start(
                    out=xt,
                    in_=xT[bass.ds(t_expr, 1)].rearrange("o k p b -> p (o k) b"))
                gt = emit_mms(xt, KX, s_wih0, s_whh0, s_b0, h_read)
                return gt

            def slot(t_l1, cur):
                # pipelined slot: L0 runs step t_l1+1, L1 runs step t_l1.
                # hT0s[cur] holds h0(t_l1); L0 writes h0(t_l1+1) to the
                # other buffer.
                gt0 = l0_step(t_l1 + 1, hT0s[cur])
                hT0v = hT0s[cur].rearrange("p (k h) -> p k h", h=128)
                gt1 = emit_mms(hT0v, KH, s_wih1, s_whh1, s_b1, hT1)
                emit_cell(gt0, hT0s[1 - cur], c0)
                emit_cell(gt1, hT1, c1)

            # prologue: L0 step 0 reads zeros (buffer 1), writes h0(0) to
            # buffer 0
            gt = l0_step(0, hT0s[1])
            emit_cell(gt, hT0s[0], c0)

            loop_steps = t_steps - 1
            n_iter = loop_steps // unroll
            if n_iter > 1:
                with tc.For_i(0, n_iter,
                              hint_engines=(mybir.EngineType.PE,)) as iv:
                    for u in range(unroll):
                        slot(iv * unroll + u, u % 2)
            else:
                for t_i in range(n_iter * unroll):
                    slot(t_i, t_i % 2)
            for t_i in range(n_iter * unroll, loop_steps):
                slot(t_i, t_i % 2)

            # epilogue: L1 step t_steps-1 reads h0(t_steps-1) at parity
            # (t_steps-1) % 2
            hT0v = hT0s[(t_steps - 1) % 2].rearrange("p (k h) -> p k h", h=128)
            gt1 = emit_mms(hT0v, KH, s_wih1, s_whh1, s_b1, hT1)
            emit_cell(gt1, hT1, c1)

            # FC partial: y = h1_final @ W_fc_dir.T  ([B, OUT])
            yps = gp.tile([B, OUT], F32, tag="gt")
            for k in range(KH):
                nc.tensor.matmul(yps, hT1[:, k * 128:(k + 1) * 128],
                                 s_wfc[:, k, :],
                                 start=(k == 0), stop=(k == KH - 1))
            ysb = st.tile([B, OUT], F32)
            nc.scalar.activation(out=ysb, in_=yps, func=AF.Copy)
            nc.sync.dma_start(out=y, in_=ysb)

    nc.compile()
    return nc


def host_inputs(x, W_ih0, W_hh0, b_ih0, b_hh0, W_ih1, W_hh1, b_ih1, b_hh1,
                W_fc, b_fc, t_steps=T):
    """Build the per-core in_maps (fwd on even cores, rev on odd cores)."""
    perm = gate_perm()

    def wprep(w, ktiles):
        # w: [4H, K_in] -> permuted-gate W^T as [ktiles, 128, G]
        wt = np.ascontiguousarray(w[perm].T.astype(np.float32))
        return np.ascontiguousarray(wt.reshape(ktiles, 128, G))

    wih0 = wprep(W_ih0, KX)
    whh0 = wprep(W_hh0, KH)
    wih1 = wprep(W_ih1, KH)
    whh1 = wprep(W_hh1, KH)
    b0 = np.ascontiguousarray((b_ih0 + b_hh0)[perm].reshape(NB, 512)
                              .astype(np.float32))
    b1 = np.ascontiguousarray((b_ih1 + b_hh1)[perm].reshape(NB, 512)
                              .astype(np.float32))
    ident = np.eye(128, dtype=np.float32)

    # x: [B, T, IN] -> x^T per step [T, KX, 128, B]
    xTf = np.ascontiguousarray(
        x.transpose(1, 2, 0).reshape(t_steps, KX, 128, B).astype(np.float32))
    rev_idx = (-np.arange(t_steps)) % t_steps
    xTr = np.ascontiguousarray(xTf[rev_idx])

    wfc_f = np.ascontiguousarray(
        W_fc[:, :H].T.astype(np.float32).reshape(KH, 128, OUT))
    wfc_r = np.ascontiguousarray(
        W_fc[:, H:].T.astype(np.float32).reshape(KH, 128, OUT))

    common = dict(wih0=wih0, whh0=whh0, wih1=wih1, whh1=whh1,
                  b0=b0, b1=b1, ident=ident,
                  ones=np.ones((1, 128), dtype=np.float32),
                  zeros=np.zeros((128, H), dtype=np.float32))
    fwd = dict(common, xT=xTf, wfc=wfc_f)
    rev = dict(common, xT=xTr, wfc=wfc_r)
    return [dict(fwd) if i % 2 == 0 else dict(rev) for i in range(N_CORES)]


_NC_CACHE = {}


def kernel(x, W_ih0, W_hh0, b_ih0, b_hh0, W_ih1, W_hh1, b_ih1, b_hh1,
           W_fc, b_fc):
    x = np.asarray(x, dtype=np.float32)
    args = [np.asarray(a, dtype=np.float32) for a in
            (W_ih0, W_hh0, b_ih0, b_hh0, W_ih1, W_hh1, b_ih1, b_hh1,
             W_fc, b_fc)]
    in_maps = host_inputs(x, *args)
    key = (T,)
    if key not in _NC_CACHE:
        _NC_CACHE[key] = build(T)
    nc = _NC_CACHE[key]
    res = bass_utils.run_bass_kernel_spmd(nc, in_maps,
                                          core_ids=list(range(N_CORES)))
    y = res.results[0]["y"] + res.results[1]["y"] + args[9][None, :]
    return y.astype(np.float32)
